# revision 2
# baseline (speedup 1.0000x reference)
"""Trainium2 Bass kernel for nn_DinoGazeSpade (segment_reduce + SPADE stack).

Layout: 8 cores; image k = core//2; each core computes rows [16h, 16h+16) of
the 32x32 grid (h = core%2). Cross-core: 3 pairwise AllReduces of LayerNorm
partial stats. Heavy convs in fp16 matmuls, fp32 accumulate.

Key algebra:
  - painted map (448x448) never materialized: bilinear 448->32 samples exactly
    4 seg pixels per output at weight 1/4, so sm = per-segment means avg[64,384]
    gathered through count matrices; scatter_mean and gather are both matmuls
    against one-hot masks built on device via is_equal(iota, ids).
  - SPADE0's wb conv (128->1536) folded through conv0_w (1x1, 1536->8) on the
    host into a 128->8 conv. Same for SPADE1/2 wb convs.
  - LN stats via bn_stats/bn_aggr; rsqrt as exp(-0.5*ln(var+eps)) so every
    activation fits one ACT table set (no table reloads).
"""
import os
import numpy as np
from contextlib import ExitStack

import concourse.bass as bass
import concourse.mybir as mybir
import concourse.tile as tile
from concourse import bacc
from concourse.bass_utils import run_bass_kernel_spmd
from concourse.masks import make_identity

f32 = mybir.dt.float32
f16 = mybir.dt.float16
AF = mybir.ActivationFunctionType
ALU = mybir.AluOpType
AX = mybir.AxisListType

NSEG = 64
B, Cd, Hp, Wp, H, W, Cm, HID = 4, 384, 32, 32, 448, 448, 1536, 128
NPOS = Hp * Wp          # 1024
HROWS = 16              # rows per core
SMR = HROWS + 4         # sm rows incl 2-halo each side = 20
HR = HROWS + 2          # h rows incl 1-halo each side = 18
SMW = 34                # padded width

LAST_RESULTS = None  # set by kernel() for test harness introspection

_BUILT = None


def _softplus(nc, pool, psum_in, bias_ap, out_tile, p, n):
    """out = softplus(psum_in + bias) = relu(z) + ln(1+exp(-|z|)) exactly."""
    t_abs = pool.tile([p, n], f32, tag="sp_abs")
    nc.scalar.activation(out=t_abs, in_=psum_in, func=AF.Abs, bias=bias_ap)
    t_exp = pool.tile([p, n], f32, tag="sp_exp")
    nc.scalar.activation(out=t_exp, in_=t_abs, func=AF.Exp, scale=-1.0)
    t_ln = pool.tile([p, n], f32, tag="sp_ln")
    nc.scalar.activation(out=t_ln, in_=t_exp, func=AF.Ln, bias=1.0)
    t_relu = pool.tile([p, n], f32, tag="sp_relu")
    nc.scalar.activation(out=t_relu, in_=psum_in, func=AF.Relu, bias=bias_ap)
    nc.vector.tensor_tensor(out=out_tile, in0=t_ln, in1=t_relu, op=ALU.add)


def _ln_finish(nc, pool, pst, work, n_inst, st_l, st_g, stg, gid):
    """pst holds partial [2,1] = (sum of per-partition means, sum of E[x^2]).
    AllReduce over the pair, then r = 1/sqrt(var+eps) and -mu*r -> work[:,5:7].
    n_inst = total partition-instances across the pair (equal counts each)."""
    st_sb = pool.tile([2, 1], f32, tag=f"st_sb{gid}")
    nc.scalar.copy(st_sb, pst)
    nc.sync.dma_start(out=st_l[:], in_=st_sb[0:2, 0:1])
    nc.gpsimd.collective_compute(
        "AllReduce", ALU.add,
        replica_groups=[[0, 1], [2, 3], [4, 5], [6, 7]],
        ins=[st_l[:]], outs=[st_g[:]],
    )
    nc.sync.dma_start(out=stg, in_=st_g[None, :])
    inv = 1.0 / n_inst
    nc.vector.tensor_scalar_mul(work[:, 0:2], stg[:, 0:2], inv)            # mu, E[x^2]
    nc.vector.tensor_tensor(out=work[:, 2:3], in0=work[:, 0:1], in1=work[:, 0:1], op=ALU.mult)
    nc.vector.tensor_tensor(out=work[:, 3:4], in0=work[:, 1:2], in1=work[:, 2:3], op=ALU.subtract)
    # r = exp(-0.5 * ln(var + eps)); ln/exp live in the same ACT table set
    nc.scalar.activation(out=work[:, 4:5], in_=work[:, 3:4], func=AF.Ln, bias=1e-12)
    nc.scalar.activation(out=work[:, 5:6], in_=work[:, 4:5], func=AF.Exp, scale=-0.5)
    nc.vector.tensor_tensor(out=work[:, 7:8], in0=work[:, 0:1], in1=work[:, 5:6], op=ALU.mult)
    nc.vector.tensor_scalar_mul(work[:, 6:7], work[:, 7:8], -1.0)          # -mu*r
    # work[:, 5:7] = [r, -mu*r]


def _bn_partial(nc, pool, src, p, nchunks, tag):
    """bn_stats over src[p, nchunks, 512] -> mv[p,2]=(mean,var); convert var to
    E[x^2] in place; return mv for use as matmul lhsT [p, 2]."""
    bno = pool.tile([p, nchunks, 6], f32, tag=f"bno{tag}")
    for kc in range(nchunks):
        nc.vector.bn_stats(out=bno[:, kc, :], in_=src[:, kc, :])
    mv = pool.tile([p, 2], f32, tag=f"mv{tag}")
    nc.vector.bn_aggr(out=mv, in_=bno)
    m2 = pool.tile([p, 1], f32, tag=f"m2{tag}")
    nc.vector.tensor_tensor(out=m2, in0=mv[:, 0:1], in1=mv[:, 0:1], op=ALU.mult)
    nc.vector.tensor_tensor(out=mv[:, 1:2], in0=mv[:, 1:2], in1=m2, op=ALU.add)
    return mv


def _build_nc():
    nc = bacc.Bacc("TRN2", num_devices=8)

    for val in (1e-12,):
        t = nc.alloc_sbuf_tensor(f"const-float32-{val}", [128, 1], f32)
        nc.gpsimd.memset(t.ap(), val)
        nc.const_aps.aps[(f32, val)] = t.ap()
    nc.all_engine_barrier()

    # ---------------- DRAM I/O ----------------
    d_x = nc.dram_tensor("x", [128, 12, 512], f16, kind="ExternalInput")
    d_ft = nc.dram_tensor("ft", [128, 8, 384], f16, kind="ExternalInput")
    d_ids = nc.dram_tensor("ids", [128, 8], f32, kind="ExternalInput")
    d_cid = nc.dram_tensor("cid", [128, 5, 4], f32, kind="ExternalInput")
    d_hmask = nc.dram_tensor("hmask", [HR], f16, kind="ExternalInput")
    d_ws = nc.dram_tensor("ws", [128, 3, 3, 9, 128], f16, kind="ExternalInput")
    d_wg = nc.dram_tensor("wg", [128, 12, 9, 128], f16, kind="ExternalInput")
    d_wf0 = nc.dram_tensor("wf0", [128, 9, 8], f16, kind="ExternalInput")
    d_wg1 = nc.dram_tensor("wg1", [128, 9, 8], f16, kind="ExternalInput")
    d_wf1 = nc.dram_tensor("wf1", [128, 9, 16], f16, kind="ExternalInput")
    d_wg2 = nc.dram_tensor("wg2", [128, 9, 16], f16, kind="ExternalInput")
    d_wf2 = nc.dram_tensor("wf2", [128, 9, 1], f16, kind="ExternalInput")
    d_w0t = nc.dram_tensor("w0t", [128, 12, 8], f16, kind="ExternalInput")
    d_w1t = nc.dram_tensor("w1t", [8, 16], f16, kind="ExternalInput")
    d_w2t = nc.dram_tensor("w2t", [16, 1], f16, kind="ExternalInput")
    d_bs = nc.dram_tensor("bs", [128, 3], f32, kind="ExternalInput")
    d_gb0 = nc.dram_tensor("gb0", [128, 12], f32, kind="ExternalInput")
    d_gb1 = nc.dram_tensor("gb1", [8], f32, kind="ExternalInput")
    d_gb2 = nc.dram_tensor("gb2", [16], f32, kind="ExternalInput")
    d_b0f = nc.dram_tensor("b0f", [8], f32, kind="ExternalInput")
    d_b1f = nc.dram_tensor("b1f", [16], f32, kind="ExternalInput")
    d_b2f = nc.dram_tensor("b2f", [1], f32, kind="ExternalInput")
    d_out = nc.dram_tensor("out_half", [512], f32, kind="ExternalOutput")

    st0_l = nc.dram_tensor("st0_l", [2], f32)
    st0_g = nc.dram_tensor("st0_g", [2], f32)
    st1_l = nc.dram_tensor("st1_l", [2], f32)
    st1_g = nc.dram_tensor("st1_g", [2], f32)
    st2_l = nc.dram_tensor("st2_l", [2], f32)
    st2_g = nc.dram_tensor("st2_g", [2], f32)

    with ExitStack() as ctx:
        tc = ctx.enter_context(tile.TileContext(nc, num_cores=8))
        cpool = ctx.enter_context(tc.tile_pool(name="consts", bufs=1))
        dpool = ctx.enter_context(tc.tile_pool(name="data", bufs=1))
        spool = ctx.enter_context(tc.tile_pool(name="small", bufs=1))
        ps = ctx.enter_context(tc.tile_pool(name="ps", bufs=1, space="PSUM"))

        # --------- DMA order: tiny aux -> feats -> ws0 -> x -> wg -> ws12 ---
        idst = cpool.tile([128, 8], f32)
        nc.sync.dma_start(out=idst, in_=d_ids[:, :])
        cidt = cpool.tile([128, 5, 4], f32)
        nc.sync.dma_start(out=cidt, in_=d_cid[:, :, :])
        bs_t = cpool.tile([128, 3], f32)
        nc.sync.dma_start(out=bs_t, in_=d_bs[:, :])
        gb0_t = cpool.tile([128, 12], f32)
        nc.sync.dma_start(out=gb0_t, in_=d_gb0[:, :])
        gb1_t = cpool.tile([8, 1], f32)
        nc.sync.dma_start(out=gb1_t, in_=d_gb1[:, None])
        gb2_t = cpool.tile([16, 1], f32)
        nc.sync.dma_start(out=gb2_t, in_=d_gb2[:, None])
        b0f_t = cpool.tile([8, 1], f32)
        nc.sync.dma_start(out=b0f_t, in_=d_b0f[:, None])
        b1f_t = cpool.tile([16, 1], f32)
        nc.sync.dma_start(out=b1f_t, in_=d_b1f[:, None])
        b2f_t = cpool.tile([1, 1], f32)
        nc.sync.dma_start(out=b2f_t, in_=d_b2f[:, None])
        w0t_t = cpool.tile([128, 12, 8], f16)
        nc.sync.dma_start(out=w0t_t, in_=d_w0t[:, :, :])
        w1t_t = cpool.tile([8, 16], f16)
        nc.sync.dma_start(out=w1t_t, in_=d_w1t[:, :])
        w2t_t = cpool.tile([16, 1], f16)
        nc.sync.dma_start(out=w2t_t, in_=d_w2t[:, :])
        wf0_t = cpool.tile([128, 9, 8], f16)
        nc.sync.dma_start(out=wf0_t, in_=d_wf0[:, :, :])
        wg1_t = cpool.tile([128, 9, 8], f16)
        nc.sync.dma_start(out=wg1_t, in_=d_wg1[:, :, :])
        wf1_t = cpool.tile([128, 9, 16], f16)
        nc.sync.dma_start(out=wf1_t, in_=d_wf1[:, :, :])
        wg2_t = cpool.tile([128, 9, 16], f16)
        nc.sync.dma_start(out=wg2_t, in_=d_wg2[:, :, :])
        wf2_t = cpool.tile([128, 9, 1], f16)
        nc.sync.dma_start(out=wf2_t, in_=d_wf2[:, :, :])
        hmask_bc = cpool.tile([128, HR], f16)
        nc.gpsimd.dma_start(out=hmask_bc, in_=d_hmask[None, :].to_broadcast([128, HR]))

        feats = dpool.tile([128, 8, 385], f16)
        nc.sync.dma_start(out=feats[:, :, 0:384], in_=d_ft[:, :, :])
        nc.gpsimd.memset(feats[:, :, 384:385], 1.0)

        ws_t = cpool.tile([128, 3, 3, 9, 128], f16)
        nc.sync.dma_start(out=ws_t[:, 0:1], in_=d_ws[:, 0:1])     # s0_ws first

        xt = dpool.tile([128, 12, 512], f16)
        nc.sync.dma_start(out=xt, in_=d_x[:, :, :])

        wg_t = cpool.tile([128, 12, 9, 128], f16)
        for kc in range(12):
            nc.sync.dma_start(out=wg_t[:, kc], in_=d_wg[:, kc])

        nc.sync.dma_start(out=ws_t[:, 1:3], in_=d_ws[:, 1:3])     # s1/s2_ws later

        iot = cpool.tile([128, 64], f32)
        nc.gpsimd.iota(iot, pattern=[[1, 64]], base=0, channel_multiplier=0,
                       allow_small_or_imprecise_dtypes=True)
        ident = cpool.tile([128, 128], f16)
        make_identity(nc, ident)
        ones_col = cpool.tile([128, 1], f32)
        nc.gpsimd.memset(ones_col, 1.0)
        ones_row = cpool.tile([1, 128], f32)
        nc.gpsimd.memset(ones_row, 1.0)

        # ---------------- segment means avg' [64, 384] ----------------
        oh_t = dpool.tile([128, 8, 64], f16)
        for qc in range(8):
            nc.vector.tensor_scalar(out=oh_t[:, qc, :], in0=iot,
                                    scalar1=idst[:, qc:qc + 1], scalar2=None,
                                    op0=ALU.is_equal)
        psums = ps.tile([64, 385], f32, tag="ps_sums", bufs=1)
        for qc in range(8):
            nc.tensor.matmul(psums, oh_t[:, qc, :], feats[:, qc, :],
                             start=(qc == 0), stop=(qc == 7))
        cnt4 = spool.tile([64, 1], f32, tag="cnt4")
        nc.vector.tensor_scalar(out=cnt4, in0=psums[:, 384:385], scalar1=1.0,
                                scalar2=4.0, op0=ALU.max, op1=ALU.mult)
        recip4 = spool.tile([64, 1], f32, tag="recip4")
        nc.vector.reciprocal(out=recip4, in_=cnt4)
        avg_t = dpool.tile([64, 384], f16)
        nc.vector.tensor_scalar_mul(avg_t, psums[:, 0:384], recip4[:, 0:1])

        # ---------------- G masks -> Gr [64, 640] ----------------
        gacc = dpool.tile([128, 5, 64], f16)
        gtmp = dpool.tile([128, 64], f16)
        for jc in range(5):
            nc.vector.tensor_scalar(out=gacc[:, jc, :], in0=iot,
                                    scalar1=cidt[:, jc, 0:1], scalar2=None,
                                    op0=ALU.is_equal)
            for corner in range(1, 4):
                nc.vector.tensor_scalar(out=gtmp, in0=iot,
                                        scalar1=cidt[:, jc, corner:corner + 1],
                                        scalar2=None, op0=ALU.is_equal)
                nc.vector.tensor_tensor(out=gacc[:, jc, :], in0=gacc[:, jc, :],
                                        in1=gtmp, op=ALU.add)
        gr_t = dpool.tile([64, 640], f16)
        for jc in range(5):
            ptr = ps.tile([64, 128], f16, tag="ps_tr", bufs=1)
            nc.tensor.transpose(ptr, gacc[:, jc, :], ident)
            nc.scalar.copy(gr_t[:, jc * 128:(jc + 1) * 128], ptr)

        # ---------------- sm ----------------
        sm_pad = dpool.tile([128, 3, SMR, SMW], f16)
        nc.gpsimd.memset(sm_pad, 0.0)
        for mc in range(3):
            for nch in range(2):
                psm = ps.tile([128, 320], f32, tag="ps_main", bufs=3)
                nc.tensor.matmul(psm, avg_t[:, mc * 128:(mc + 1) * 128],
                                 gr_t[:, nch * 320:(nch + 1) * 320],
                                 start=True, stop=True)
                nc.scalar.copy(sm_pad[:, mc, nch * 10:(nch + 1) * 10, 1:33],
                               psm.rearrange("p (r c) -> p r c", c=32))

        # ---------------- h conv helper ----------------
        def h_conv(cv):
            hp = dpool.tile([128, HR, SMW], f16, tag=f"hpad{cv}", name=f"hpad{cv}")
            nc.gpsimd.memset(hp, 0.0)
            for nch in range(2):
                psh = ps.tile([128, 9 * 32], f32, tag="ps_main", bufs=3, name=f"psh{cv}{nch}")
                first = True
                for kc in range(3):
                    for t in range(9):
                        dy, dx = t // 3, t % 3
                        r0 = nch * 9 + dy
                        nc.tensor.matmul(
                            psh, ws_t[:, cv, kc, t, :],
                            sm_pad[:, kc, r0:r0 + 9, dx:dx + 32],
                            start=first, stop=(kc == 2 and t == 8))
                        first = False
                nc.scalar.activation(
                    out=hp[:, nch * 9:(nch + 1) * 9, 1:33],
                    in_=psh.rearrange("p (r c) -> p r c", c=32),
                    func=AF.Relu, bias=bs_t[:, cv:cv + 1])
            nc.vector.tensor_tensor(
                out=hp, in0=hp,
                in1=hmask_bc[:, :, None].to_broadcast([128, HR, SMW]),
                op=ALU.mult)
            return hp

        h0p = h_conv(0)

        # ---------------- LN0 stats (bn) + collective + xn ----------------
        mv0 = _bn_partial(nc, dpool, xt, 128, 12, "0")
        pst0 = ps.tile([2, 1], f32, tag="ps_tiny", bufs=1, name="pst0")
        nc.tensor.matmul(pst0, mv0, ones_col, start=True, stop=True)
        stg0 = spool.tile([1, 2], f32, tag="stg0")
        work0 = spool.tile([1, 8], f32, tag="work0")
        _ln_finish(nc, spool, pst0, work0, 256.0, st0_l, st0_g, stg0, 0)
        pbc0 = ps.tile([128, 2], f32, tag="ps_tiny", bufs=1, name="pbc0")
        nc.tensor.matmul(pbc0, ones_row, work0[:, 5:7], start=True, stop=True)
        rbc0 = spool.tile([128, 2], f32, tag="rbc0")
        nc.scalar.copy(rbc0, pbc0)
        xn = dpool.tile([128, 12, 512], f16)
        nc.vector.tensor_scalar(out=xn, in0=xt, scalar1=rbc0[:, 0:1],
                                scalar2=rbc0[:, 1:2], op0=ALU.mult, op1=ALU.add)

        # ---------------- g0 + mod; out0 ----------------
        mod = dpool.tile([128, 12, 512], f16)
        for kc in range(12):
            psg = ps.tile([128, 512], f32, tag="ps_main", bufs=3, name=f"psg{kc}")
            for t in range(9):
                dy, dx = t // 3, t % 3
                nc.tensor.matmul(psg, wg_t[:, kc, t, :],
                                 h0p[:, dy:dy + 16, dx:dx + 32],
                                 start=(t == 0), stop=(t == 8))
            gp1 = dpool.tile([128, 512], f16, tag="gp1", name=f"gp1_{kc}")
            nc.scalar.activation(out=gp1, in_=psg, func=AF.Identity,
                                 bias=gb0_t[:, kc:kc + 1])
            nc.vector.tensor_tensor(out=mod[:, kc, :], in0=xn[:, kc, :],
                                    in1=gp1, op=ALU.mult)

        pso0 = ps.tile([8, 512], f32, tag="ps_out", bufs=2, name="pso0")
        for kc in range(12):
            nc.tensor.matmul(pso0, w0t_t[:, kc, :], mod[:, kc, :],
                             start=(kc == 0), stop=False)
        for t in range(9):
            dy, dx = t // 3, t % 3
            nc.tensor.matmul(pso0, wf0_t[:, t, :], h0p[:, dy:dy + 16, dx:dx + 32],
                             start=False, stop=(t == 8))
        out0 = dpool.tile([8, 512], f32)
        _softplus(nc, dpool, pso0, b0f_t[:, 0:1], out0, 8, 512)

        # ---------------- LN1 stats + collective (h1 conv fills the gap) ----
        mv1 = _bn_partial(nc, spool, out0[:, None, :], 8, 1, "1")
        pst1 = ps.tile([2, 1], f32, tag="ps_tiny", bufs=1, name="pst1")
        nc.tensor.matmul(pst1, mv1, ones_col[0:8, :], start=True, stop=True)
        stg1 = spool.tile([1, 2], f32, tag="stg1")
        work1 = spool.tile([1, 8], f32, tag="work1")
        _ln_finish(nc, spool, pst1, work1, 16.0, st1_l, st1_g, stg1, 1)

        h1p = h_conv(1)   # overlaps the LN1 collective on PE

        pbc1 = ps.tile([8, 2], f32, tag="ps_tiny", bufs=1, name="pbc1")
        nc.tensor.matmul(pbc1, ones_row[:, 0:8], work1[:, 5:7], start=True, stop=True)
        rbc1 = spool.tile([8, 2], f32, tag="rbc1")
        nc.scalar.copy(rbc1, pbc1)

        psg1 = ps.tile([8, 512], f32, tag="ps_out", bufs=2, name="psg1")
        for t in range(9):
            dy, dx = t // 3, t % 3
            nc.tensor.matmul(psg1, wg1_t[:, t, :], h1p[:, dy:dy + 16, dx:dx + 32],
                             start=(t == 0), stop=(t == 8))
        xn1 = spool.tile([8, 512], f16, tag="xn1")
        nc.vector.tensor_scalar(out=xn1, in0=out0, scalar1=rbc1[:, 0:1],
                                scalar2=rbc1[:, 1:2], op0=ALU.mult, op1=ALU.add)
        gp11 = spool.tile([8, 512], f16, tag="gp11")
        nc.scalar.activation(out=gp11, in_=psg1, func=AF.Identity,
                             bias=gb1_t[:, 0:1])
        mod1 = spool.tile([8, 512], f16, tag="mod1")
        nc.vector.tensor_tensor(out=mod1, in0=xn1, in1=gp11, op=ALU.mult)

        pso1 = ps.tile([16, 512], f32, tag="ps_out", bufs=2, name="pso1")
        nc.tensor.matmul(pso1, w1t_t, mod1, start=True, stop=False)
        for t in range(9):
            dy, dx = t // 3, t % 3
            nc.tensor.matmul(pso1, wf1_t[:, t, :], h1p[:, dy:dy + 16, dx:dx + 32],
                             start=False, stop=(t == 8))
        out1 = dpool.tile([16, 512], f32)
        _softplus(nc, dpool, pso1, b1f_t[:, 0:1], out1, 16, 512)

        # ---------------- LN2 stats + collective (h2 conv fills the gap) ----
        mv2 = _bn_partial(nc, spool, out1[:, None, :], 16, 1, "2")
        pst2 = ps.tile([2, 1], f32, tag="ps_tiny", bufs=1, name="pst2")
        nc.tensor.matmul(pst2, mv2, ones_col[0:16, :], start=True, stop=True)
        stg2 = spool.tile([1, 2], f32, tag="stg2")
        work2 = spool.tile([1, 8], f32, tag="work2")
        _ln_finish(nc, spool, pst2, work2, 32.0, st2_l, st2_g, stg2, 2)

        h2p = h_conv(2)   # overlaps the LN2 collective on PE

        pbc2 = ps.tile([16, 2], f32, tag="ps_tiny", bufs=1, name="pbc2")
        nc.tensor.matmul(pbc2, ones_row[:, 0:16], work2[:, 5:7], start=True, stop=True)
        rbc2 = spool.tile([16, 2], f32, tag="rbc2")
        nc.scalar.copy(rbc2, pbc2)

        psg2 = ps.tile([16, 512], f32, tag="ps_out", bufs=2, name="psg2")
        for t in range(9):
            dy, dx = t // 3, t % 3
            nc.tensor.matmul(psg2, wg2_t[:, t, :], h2p[:, dy:dy + 16, dx:dx + 32],
                             start=(t == 0), stop=(t == 8))
        xn2 = spool.tile([16, 512], f16, tag="xn2")
        nc.vector.tensor_scalar(out=xn2, in0=out1, scalar1=rbc2[:, 0:1],
                                scalar2=rbc2[:, 1:2], op0=ALU.mult, op1=ALU.add)
        gp12 = spool.tile([16, 512], f16, tag="gp12")
        nc.scalar.activation(out=gp12, in_=psg2, func=AF.Identity,
                             bias=gb2_t[:, 0:1])
        mod2 = spool.tile([16, 512], f16, tag="mod2")
        nc.vector.tensor_tensor(out=mod2, in0=xn2, in1=gp12, op=ALU.mult)

        pso2 = ps.tile([1, 512], f32, tag="ps_out", bufs=2, name="pso2")
        nc.tensor.matmul(pso2, w2t_t, mod2, start=True, stop=False)
        for t in range(9):
            dy, dx = t // 3, t % 3
            nc.tensor.matmul(pso2, wf2_t[:, t, :], h2p[:, dy:dy + 16, dx:dx + 32],
                             start=False, stop=(t == 8))
        final = dpool.tile([1, 512], f32)
        _softplus(nc, dpool, pso2, b2f_t[:, 0:1], final, 1, 512)
        nc.sync.dma_start(out=d_out[:], in_=final[0:1, :])

    nc.compile()
    return nc


def _host_prep(inputs):
    """Build per-core in_maps (host work: slicing, layout, small weight folds)."""
    x_main = np.asarray(inputs["x_main"], np.float32)
    f_sem = np.asarray(inputs["f_sem"], np.float32)
    seg = np.asarray(inputs["seg_mask"])

    def lhsT9(w):  # [O, I, 3, 3] -> [I, 9, O]
        return np.ascontiguousarray(w.transpose(1, 2, 3, 0).reshape(w.shape[1], 9, w.shape[0]))

    ws_stack = np.stack([inputs["s0_ws"], inputs["s1_ws"], inputs["s2_ws"]])  # [3,128,384,3,3]
    ws_r = ws_stack.reshape(3, 128, 3, 128, 3, 3)          # cv, o, kc, i, ky, kx
    WS = np.ascontiguousarray(ws_r.transpose(3, 0, 2, 4, 5, 1)
                              .reshape(128, 3, 3, 9, 128)).astype(np.float16)
    wg0 = np.asarray(inputs["s0_wg"], np.float32)          # [1536, 128, 3, 3]
    WG = np.ascontiguousarray(
        wg0.reshape(12, 128, 128, 3, 3).transpose(2, 0, 3, 4, 1)
        .reshape(128, 12, 9, 128)).astype(np.float16)
    wf0 = np.einsum("oc,cikl->oikl", np.asarray(inputs["conv0_w"], np.float64),
                    np.asarray(inputs["s0_wb"], np.float64))
    WF0 = lhsT9(wf0).astype(np.float16)
    WG1 = lhsT9(np.asarray(inputs["s1_wg"], np.float32)).astype(np.float16)
    wf1 = np.einsum("oc,cikl->oikl", np.asarray(inputs["conv1_w"], np.float64),
                    np.asarray(inputs["s1_wb"], np.float64))
    WF1 = lhsT9(wf1).astype(np.float16)
    WG2 = lhsT9(np.asarray(inputs["s2_wg"], np.float32)).astype(np.float16)
    wf2 = np.einsum("oc,cikl->oikl", np.asarray(inputs["conv2_w"], np.float64),
                    np.asarray(inputs["s2_wb"], np.float64))
    WF2 = lhsT9(wf2).astype(np.float16)
    W0T = np.ascontiguousarray(np.asarray(inputs["conv0_w"], np.float32).T
                               .reshape(12, 128, 8).transpose(1, 0, 2)).astype(np.float16)
    W1T = np.ascontiguousarray(np.asarray(inputs["conv1_w"], np.float32).T).astype(np.float16)
    W2T = np.ascontiguousarray(np.asarray(inputs["conv2_w"], np.float32).T).astype(np.float16)
    BS = np.ascontiguousarray(np.stack([inputs["s0_bs"], inputs["s1_bs"],
                                        inputs["s2_bs"]]).T).astype(np.float32)  # [128,3]
    GB0 = np.ascontiguousarray((1.0 + np.asarray(inputs["s0_bg"], np.float32))
                               .reshape(12, 128).T).astype(np.float32)           # [128,12]
    GB1 = (1.0 + np.asarray(inputs["s1_bg"], np.float32))
    GB2 = (1.0 + np.asarray(inputs["s2_bg"], np.float32))
    B0F = (np.asarray(inputs["b0"], np.float64)
           + np.asarray(inputs["conv0_w"], np.float64) @ np.asarray(inputs["s0_bb"], np.float64)
           ).astype(np.float32)
    B1F = (np.asarray(inputs["b1"], np.float64)
           + np.asarray(inputs["conv1_w"], np.float64) @ np.asarray(inputs["s1_bb"], np.float64)
           ).astype(np.float32)
    B2F = (np.asarray(inputs["b2"], np.float64)
           + np.asarray(inputs["conv2_w"], np.float64) @ np.asarray(inputs["s2_bb"], np.float64)
           ).astype(np.float32)

    shared = dict(ws=WS, wg=WG, wf0=WF0, wg1=WG1, wf1=WF1, wg2=WG2, wf2=WF2,
                  w0t=W0T, w1t=W1T, w2t=W2T, bs=BS, gb0=GB0, gb1=GB1, gb2=GB2,
                  b0f=B0F, b1f=B1F, b2f=B2F)

    in_maps = []
    for core in range(8):
        k, h = core // 2, core % 2
        r0 = HROWS * h
        X = np.ascontiguousarray(
            x_main[k, :, r0:r0 + HROWS, :].reshape(12, 128, 512).transpose(1, 0, 2)
        ).astype(np.float16)
        FT = np.ascontiguousarray(
            f_sem[k].reshape(384, NPOS).T.reshape(8, 128, 384).transpose(1, 0, 2)
        ).astype(np.float16)
        ids_flat = seg[k, ::14, ::14].astype(np.float32).reshape(NPOS)
        IDS = np.ascontiguousarray(ids_flat.reshape(8, 128).T)
        rows = np.arange(r0 - 2, r0 + HROWS + 2)          # 20 sm rows
        valid = (rows >= 0) & (rows < Hp)
        rcl = np.clip(rows, 0, Hp - 1)
        cid = np.empty((SMR, Wp, 4), np.float32)
        cols = np.arange(Wp)
        for t, (dy, dx) in enumerate([(0, 0), (0, 1), (1, 0), (1, 1)]):
            v = seg[k][np.ix_(14 * rcl + 6 + dy, 14 * cols + 6 + dx)].astype(np.float32)
            v[~valid, :] = -1.0
            cid[:, :, t] = v
        CID = np.ascontiguousarray(cid.reshape(5, 128, 4).transpose(1, 0, 2))
        hrows = np.arange(r0 - 1, r0 + HROWS + 1)
        HM = ((hrows >= 0) & (hrows < Hp)).astype(np.float16)
        in_maps.append(dict(shared, x=X, ft=FT, ids=IDS, cid=CID, hmask=HM))
    return in_maps


def kernel(**inputs):
    global _BUILT, LAST_RESULTS
    if _BUILT is None:
        _BUILT = _build_nc()
    nc = _BUILT
    in_maps = _host_prep(inputs)
    trace = bool(os.environ.get("BASS_TRACE"))
    res = run_bass_kernel_spmd(nc, in_maps, list(range(8)), trace=trace)
    LAST_RESULTS = res
    out = np.empty((B, 1, Hp, Wp), np.float32)
    for core in range(8):
        k, h = core // 2, core % 2
        out[k, 0, HROWS * h:HROWS * (h + 1), :] = \
            res.results[core]["out_half"].reshape(HROWS, Wp)
    return out
